# revision 28
# baseline (speedup 1.0000x reference)
"""Causal attention kernel for Trainium2 (Bass/Tile), 8-core SPMD.

Problem: B=2, H=16, S=2048, D=64 fp32 attention with a causal mask.
Sharding: batch*heads = 32 slices -> 4 heads per core across 8 cores.

Per-core algorithm (heads processed in pairs, stacked in partitions):
  S^T = K @ Q^T computed per kblock: a [128, 2*QT] PSUM "pair tile" holds
  both heads' scores for one kblock (head0 in cols 0..QT, head1 in
  QT..2*QT). The two QK^T matmuls are issued back-to-back with lhsT at
  partition offsets 0/64, so they land on disjoint PE row-tiles
  ((0,0)/(64,0)) and execute concurrently (2x QK throughput).

  P^T = exp(S^T / 8) is split across TWO engines to break the ScalarE
  wall (exp elems/core = 8.4M at 1 col/cycle = 60us+ on ACT alone):
   - ACT kblocks: ScalarE activation, bf16 output (exact exp).
   - DVE kblocks: ONE tensor_scalar computing the Schraudolph bit-trick
     in the bf16 bit domain: int16(round(s*2^7*log2e/8 + (127-sig)*2^7))
     bitcast as bf16 == exp with ~3% ripple (well under the 2e-2 rel_err
     budget; measured end-to-end error ~1.2e-2).
  kblocks are assigned greedily to whichever engine has less accumulated
  work. Mixed (partially masked) kblocks go to DVE; the mask multiply is
  a bf16 tensor_tensor (2x DVE mode) against a resident doubled mask
  tile [KB, 2*QT] (causal patterns generated on-chip by GpSimd).

  out^T = V_aug^T @ P^T accumulated over kblocks in PSUM (V_aug bf16
  with a ones column -> row 64 of out^T is the softmax denominator).
  Host divides by the denominator and transposes back.

  QK matmuls are float32r (1 cycle/row at N>=256); PV matmuls are bf16.
  PSUM: 3 st pair-tiles (2 banks each) + 2 accs = 8 banks, giving the
  scheduler 3 kblocks of lookahead to keep PE/ACT/DVE all busy.
"""

import sys

import numpy as np

for _p in ('/opt/trn_rl_repo', '/root/.axon_site/_ro/trn_rl_repo'):
    if _p not in sys.path:
        sys.path.append(_p)

B, H, S, D = 2, 16, 2048, 64
NCORES = 8
HPC = (B * H) // NCORES  # heads per core = 4
QT = 512                 # q tile (PSUM bank free dim)
KB = 128                 # k block (partition dim)
NQT = S // QT            # 4
NKB = S // KB            # 16
MAX_RESIDENT_MASKS = 8   # unique mask tiles kept SBUF-resident

SIGMA = 0.045
A16 = float(2**7 * np.log2(np.e) / np.sqrt(D))
B16 = float((127.0 - SIGMA) * 2**7)

# per-kblock engine cost estimates (ns) for the greedy assignment
ACT_FIXED = 200.0
ACT_CYCLE = 1 / 1.2
DVE_FIXED = 270.0
DVE_CYCLE = 1 / 0.96
TT_COST = 400.0     # bf16 mask multiply (2x DVE mode)
COPY_COST = 780.0   # [65, 512] PSUM->SBUF tensor_copy on DVE
ACT_TABLE = 1300.0

_CACHE = {}


def _plan_from_mask(mask):
    """Classify each (qtile, kblock) region of the mask.

    Returns (plan, tiles). plan[j] is a tuple of active kblocks
    (i, kind, mi, y0): kind in {'full','mixed'}, mi indexes the deduped
    mask tiles ([KB, QT], stored doubled to [KB, 2*QT] for the pair
    layout), y0 trims to q-columns >= y0 (multiple of 256; all columns
    < y0 are fully masked). Mixed kblocks are ordered first.
    """
    plan = []
    tiles = []
    tile_idx = {}
    for j in range(NQT):
        row = []
        mq = mask[j * QT:(j + 1) * QT]  # [QT, S] (q rows, k cols)
        for i in range(NKB):
            R = mq[:, i * KB:(i + 1) * KB]  # [QT, KB]
            if not R.any():
                continue
            if R.all():
                row.append((i, 'full', 0, 0))
                continue
            live = np.nonzero(R.any(axis=1))[0]
            y0 = min((int(live[0]) // 256) * 256, QT - 256)
            Mt = np.ascontiguousarray(R.T).astype(np.float32)  # [KB, QT]
            key = Mt.tobytes()
            if key not in tile_idx:
                tile_idx[key] = len(tiles)
                qq = np.arange(j * QT, (j + 1) * QT)[None, :]
                kk = np.arange(i * KB, (i + 1) * KB)[:, None]
                if np.array_equal(R.T, qq >= kk):
                    # on-chip generatable: keep where y - x - rel >= 0
                    tiles.append(('causal', i * KB - j * QT))
                else:
                    tiles.append(('data', Mt))
            row.append((i, 'mixed', tile_idx[key], y0))
        # mixed kblocks first: their longer chains start early and full
        # kblocks fill the pipeline behind them
        row.sort(key=lambda e: e[1] != 'mixed')
        plan.append(tuple(row))
    return tuple(plan), tiles


def _assign_engines(plan, tiles):
    """Exp-engine assignment: cumulative-busy greedy over ACT and DVE.

    Each kblock's exp (and each out copy) goes to whichever engine has
    the smaller projected cumulative busy time — globally balanced with
    short local runs, so both engines stay fed within the 3-tile st
    lookahead. Causal masking runs on GpSimd (affine_select on pt);
    only generic data masks cost DVE a tensor_tensor.
    Returns (assign {(pair, j, idx)}, copy_assign {(pair, j, sub)}),
    values 'act' | 'dve'.
    """
    assign = {}
    copy_assign = {}
    act_free, dve_free = ACT_TABLE, 0.0
    clock = 0.0
    for j in range(NQT):
        for pair in range(HPC // 2):
            for idx, (i, kind, mi, y0) in enumerate(plan[j]):
                cols = 2 * (QT - y0)
                clock += 2 * cols * 0.417  # QK pair + 2 PV on PE
                a_fin = max(act_free, clock) + cols * ACT_CYCLE + ACT_FIXED
                d_fin = max(dve_free, clock) + cols * DVE_CYCLE + DVE_FIXED
                if d_fin < a_fin:
                    assign[(pair, j, idx)] = 'dve'
                    dve_free = d_fin
                else:
                    assign[(pair, j, idx)] = 'act'
                    act_free = a_fin
                if kind == 'mixed':
                    fin = a_fin if assign[(pair, j, idx)] == 'act' else d_fin
                    dve_free = max(dve_free, fin) + TT_COST
            for sub in range(2):
                copy_assign[(pair, j, sub)] = 'dve'
                dve_free = max(dve_free, clock) + COPY_COST
    return assign, copy_assign


def _stack_mask_tiles(tiles):
    """Stack data mask tiles doubled into bf16 [n, KB, 2*QT]."""
    import ml_dtypes
    data = [t[1] for t in tiles if t[0] == 'data']
    if not data:
        return None
    out = np.empty((len(data), KB, 2 * QT), dtype=ml_dtypes.bfloat16)
    for i, t in enumerate(data):
        tb = t.astype(ml_dtypes.bfloat16)
        out[i, :, :QT] = tb
        out[i, :, QT:] = tb
    return out


def _build(plan, tiles, repeats=1):
    from contextlib import ExitStack

    import concourse.tile as tile
    from concourse import bacc, mybir

    f32 = mybir.dt.float32
    f32r = mybir.dt.float32r
    bf16 = mybir.dt.bfloat16
    i16 = mybir.dt.int16

    assign, copy_assign = _assign_engines(plan, tiles)

    nc = bacc.Bacc("TRN2", target_bir_lowering=False, debug=False,
                   num_devices=NCORES)

    qt_d = nc.dram_tensor("qt", [HPC // 2, 128, S], f32r, kind="ExternalInput").ap()
    kt_d = nc.dram_tensor("kt", [HPC // 2, 128, S], f32r, kind="ExternalInput").ap()
    v_d = nc.dram_tensor("v", [HPC, 128, NKB * (D + 1)], bf16,
                         kind="ExternalInput").ap()
    out_d = nc.dram_tensor("out", [HPC, D + 1, S], f32, kind="ExternalOutput").ap()
    data_idx = {}  # tile index -> position in the stacked "mt" input
    for ti, t in enumerate(tiles):
        if t[0] == 'data':
            data_idx[ti] = len(data_idx)
    n_mtiles = len(data_idx)
    if n_mtiles:
        mt_d = nc.dram_tensor("mt", [n_mtiles, KB, 2 * QT], bf16,
                              kind="ExternalInput").ap()
    resident = n_mtiles <= MAX_RESIDENT_MASKS

    with tile.TileContext(nc) as tc, ExitStack() as ctx:
        qk_pool = ctx.enter_context(tc.tile_pool(name="qk", bufs=2))
        v_pool = ctx.enter_context(tc.tile_pool(name="vp", bufs=4))
        st_pool = ctx.enter_context(tc.tile_pool(name="st", bufs=3, space="PSUM"))
        pt_pool = ctx.enter_context(tc.tile_pool(name="pt", bufs=8))
        acc_pool = ctx.enter_context(tc.tile_pool(name="acc", bufs=2, space="PSUM"))
        out_pool = ctx.enter_context(tc.tile_pool(name="ob", bufs=4))
        warm_pool = ctx.enter_context(tc.tile_pool(name="wm", bufs=1))

        # prewarm the exp table on ACT while the first DMAs are in flight
        warm = warm_pool.tile([128, 8], f32, name="warm")
        nc.vector.memset(warm[:], 0.0)
        warm_o = warm_pool.tile([128, 8], bf16, name="warmo")
        nc.scalar.activation(warm_o[:], warm[:],
                             mybir.ActivationFunctionType.Exp, scale=1.0)

        m_tiles = {}
        mt_pool = ctx.enter_context(
            tc.tile_pool(name="mt", bufs=1 if resident else 2))
        # causal-pattern masks: generated on-chip once, doubled for the
        # pair layout (same pattern in both QT halves)
        for ti, t in enumerate(tiles):
            if t[0] != 'causal':
                continue
            rel = t[1]
            m = mt_pool.tile([KB, 2 * QT], bf16, tag=f"m{ti}", name=f"m{ti}")
            nc.gpsimd.memset(m[:], 1.0)
            m3g = m[:].rearrange("p (s y) -> p s y", y=QT)
            nc.gpsimd.affine_select(
                out=m3g, in_=m3g,
                compare_op=mybir.AluOpType.is_ge,
                fill=0.0, base=-rel,
                pattern=[[0, 2], [1, QT]],
                channel_multiplier=-1)
            m_tiles[ti] = m

        def _preload_masks():
            if n_mtiles and resident:
                for ti, di in data_idx.items():
                    m = mt_pool.tile([KB, 2 * QT], bf16, tag=f"m{ti}",
                                     name=f"md{ti}")
                    nc.sync.dma_start(m[:], mt_d[di])
                    m_tiles[ti] = m

        NPAIR = HPC // 2
        for rep in range(repeats):
            # chunked loads, both pairs interleaved: compute on qtile j
            # needs only k/q chunks <= j, so the first matmuls start after
            # a few chunk DMAs
            kt_c = {}
            qt_c = {}
            v_ts = {}
            out_sbs = {}
            for c in range(NQT):
                for pair in range(NPAIR):
                    kt1 = qk_pool.tile([128, QT], f32r, tag=f"kt{pair}_{c}",
                                       name=f"kt{pair}_{c}")
                    nc.sync.dma_start(kt1[:], kt_d[pair, :, c * QT:(c + 1) * QT])
                    kt_c[pair, c] = kt1
                    qt1 = qk_pool.tile([128, QT], f32r, tag=f"qt{pair}_{c}",
                                       name=f"qt{pair}_{c}")
                    nc.sync.dma_start(qt1[:], qt_d[pair, :, c * QT:(c + 1) * QT])
                    qt_c[pair, c] = qt1
                if c == 0:
                    for h in range(HPC):
                        v_t = v_pool.tile([128, NKB * (D + 1)], bf16, tag="v",
                                          name=f"v{h}")
                        nc.sync.dma_start(v_t[:], v_d[h])
                        v_ts[h] = v_t
                        out_sbs[h] = out_pool.tile([D + 1, S], f32, tag="o",
                                                   name=f"ob{h}")
                    _preload_masks()

            # pairs interleaved at qtile granularity: independent work keeps
            # the exp engines fed through each qtile's ramp-down
            for j in range(NQT):
              for pair in range(NPAIR):
                active = plan[j]
                accs = [acc_pool.tile([D + 1, QT], f32, tag="a",
                                      name=f"acc{pair}_{j}_{sub}")
                        for sub in range(2)]
                for idx, (i, kind, mi, y0) in enumerate(active):
                    st = st_pool.tile([128, 2 * QT], f32, tag="s",
                                      name=f"st{pair}_{j}_{idx}")
                    # both heads' QK^T back-to-back: disjoint PE row-tiles
                    # (0,0)/(64,0) -> concurrent execution
                    for sub in range(2):
                        po = 64 * sub
                        nc.tensor.matmul(
                            st[:, sub * QT + y0:(sub + 1) * QT],
                            lhsT=kt_c[pair, i // 4][po:po + 64,
                                                    (i % 4) * KB:(i % 4 + 1) * KB],
                            rhs=qt_c[pair, j][po:po + 64, y0:],
                            start=True, stop=True)
                    pt = pt_pool.tile([128, 2 * QT], bf16, tag="p",
                                      name=f"pt{pair}_{j}_{idx}")
                    if y0 == 0:
                        st_ap = st[:, :]
                        pt_ap = pt[:, :]
                    else:
                        st_ap = st[:].rearrange(
                            "p (s y) -> p s y", y=QT)[:, :, y0:]
                        pt_ap = pt[:].rearrange(
                            "p (s y) -> p s y", y=QT)[:, :, y0:]
                    if assign[(pair, j, idx)] == 'act':
                        nc.scalar.activation(
                            pt_ap, st_ap,
                            mybir.ActivationFunctionType.Exp,
                            scale=float(1.0 / np.sqrt(D)))
                    else:
                        nc.vector.tensor_scalar(
                            pt_ap.bitcast(i16), st_ap, A16, B16,
                            mybir.AluOpType.mult, mybir.AluOpType.add)
                    if kind == 'mixed':
                        if mi in m_tiles:
                            m_t = m_tiles[mi]
                        else:
                            m_t = mt_pool.tile([KB, 2 * QT], bf16, tag="ms")
                            nc.sync.dma_start(m_t[:], mt_d[data_idx[mi]])
                        if y0 == 0:
                            m_ap = m_t[:, :]
                        else:
                            m_ap = m_t[:].rearrange(
                                "p (s y) -> p s y", y=QT)[:, :, y0:]
                        nc.vector.tensor_tensor(
                            pt_ap, pt_ap, m_ap, mybir.AluOpType.mult)
                    for sub in range(2):
                        nc.tensor.matmul(
                            accs[sub][:, y0:],
                            lhsT=v_ts[2 * pair + sub][:,
                                                      i * (D + 1):(i + 1) * (D + 1)],
                            rhs=pt[:, sub * QT + y0:(sub + 1) * QT],
                            start=(idx == 0),
                            stop=(idx == len(active) - 1))
                for sub in range(2):
                    osl = out_sbs[2 * pair + sub][:, j * QT:(j + 1) * QT]
                    if not active:
                        nc.vector.memset(osl, 0.0)
                    elif copy_assign[(pair, j, sub)] == 'act':
                        nc.scalar.activation(
                            osl, accs[sub][:],
                            mybir.ActivationFunctionType.Copy, scale=1.0)
                    else:
                        nc.vector.tensor_copy(osl, accs[sub][:])
                    nc.sync.dma_start(
                        out_d[2 * pair + sub, :, j * QT:(j + 1) * QT], osl)

    nc.compile()
    return nc


def _get_nc(mask):
    key = mask.tobytes()
    if key not in _CACHE:
        plan, mtiles = _plan_from_mask(mask)
        nc = _build(plan, mtiles)
        _CACHE[key] = (nc, mtiles)
    return _CACHE[key]


def kernel(q, k, v, mask, _trace=False):
    import ml_dtypes
    from concourse.bass_utils import run_bass_kernel_spmd

    mask = np.asarray(mask).astype(bool)
    q = np.asarray(q, dtype=np.float32).reshape(B * H, S, D)
    k = np.asarray(k, dtype=np.float32).reshape(B * H, S, D)
    v = np.asarray(v, dtype=np.float32).reshape(B * H, S, D)

    nc, mtiles = _get_nc(mask)
    mt = _stack_mask_tiles(mtiles)

    in_maps = []
    for c in range(NCORES):
        sl = slice(HPC * c, HPC * (c + 1))
        qc = np.ascontiguousarray(q[sl].transpose(0, 2, 1)).reshape(HPC // 2, 128, S)
        kc = np.ascontiguousarray(k[sl].transpose(0, 2, 1)).reshape(HPC // 2, 128, S)
        vc = np.concatenate(
            [v[sl], np.ones((HPC, S, 1), dtype=np.float32)], axis=2)
        # swizzle to the SBUF layout: [HPC, partition, kblock*(D+1)]
        vc = vc.reshape(HPC, NKB, KB, D + 1).transpose(0, 2, 1, 3)
        vc = np.ascontiguousarray(vc).reshape(HPC, KB, NKB * (D + 1))
        m = {"qt": qc, "kt": kc, "v": vc.astype(ml_dtypes.bfloat16)}
        if mt is not None:
            m["mt"] = mt
        in_maps.append(m)

    res = run_bass_kernel_spmd(nc, in_maps, core_ids=list(range(NCORES)),
                               trace=_trace)

    outs = []
    for c in range(NCORES):
        o = res.results[c]["out"]  # [HPC, D+1, S]
        num = o[:, :D, :]
        den = o[:, D:D + 1, :]
        with np.errstate(invalid='ignore', divide='ignore'):
            outs.append((num / den).transpose(0, 2, 1))  # [HPC, S, D]
    full = np.concatenate(outs, axis=0).reshape(B, H, S, D).astype(np.float32)
    if _trace:
        return full, res
    return full


# revision 32
# speedup vs baseline: 1.1836x; 1.1836x over previous
"""Causal attention kernel for Trainium2 (Bass/Tile), 8-core SPMD.

Problem: B=2, H=16, S=2048, D=64 fp32 attention with a causal mask.
Sharding: batch*heads = 32 slices -> 4 heads per core across 8 cores.

Per-core algorithm (heads processed in pairs, stacked in partitions):
  S^T = K @ Q^T computed per kblock: a [128, 2*QT] PSUM "pair tile" holds
  both heads' scores for one kblock (head0 in cols 0..QT, head1 in
  QT..2*QT). The two QK^T matmuls are issued back-to-back with lhsT at
  partition offsets 0/64, so they land on disjoint PE row-tiles
  ((0,0)/(64,0)) and execute concurrently (2x QK throughput).

  P^T = exp(S^T / 8) is split across TWO engines to break the ScalarE
  wall (exp elems/core = 8.4M at 1 col/cycle = 60us+ on ACT alone):
   - ACT kblocks: ScalarE activation, bf16 output (exact exp).
   - DVE kblocks: ONE tensor_scalar computing the Schraudolph bit-trick
     in the bf16 bit domain: int16(round(s*2^7*log2e/8 + (127-sig)*2^7))
     bitcast as bf16 == exp with ~3% ripple (well under the 2e-2 rel_err
     budget; measured end-to-end error ~1.2e-2).
  kblocks are assigned greedily to whichever engine has less accumulated
  work. Mixed (partially masked) kblocks go to DVE; the mask multiply is
  a bf16 tensor_tensor (2x DVE mode) against a resident doubled mask
  tile [KB, 2*QT] (causal patterns generated on-chip by GpSimd).

  out^T = V_aug^T @ P^T accumulated over kblocks in PSUM (V_aug bf16
  with a ones column -> row 64 of out^T is the softmax denominator).
  Host divides by the denominator and transposes back.

  QK matmuls are float32r (1 cycle/row at N>=256); PV matmuls are bf16.
  PSUM: 3 st pair-tiles (2 banks each) + 2 accs = 8 banks, giving the
  scheduler 3 kblocks of lookahead to keep PE/ACT/DVE all busy.
"""

import sys

import numpy as np

for _p in ('/opt/trn_rl_repo', '/root/.axon_site/_ro/trn_rl_repo'):
    if _p not in sys.path:
        sys.path.append(_p)

B, H, S, D = 2, 16, 2048, 64
NCORES = 8
HPC = (B * H) // NCORES  # heads per core = 4
QT = 512                 # q tile (PSUM bank free dim)
KB = 128                 # k block (partition dim)
NQT = S // QT            # 4
NKB = S // KB            # 16
MAX_RESIDENT_MASKS = 8   # unique mask tiles kept SBUF-resident

SIGMA = 0.045
A16 = float(2**7 * np.log2(np.e) / np.sqrt(D))
B16 = float((127.0 - SIGMA) * 2**7)

# per-kblock engine cost estimates (ns) for the greedy assignment
ACT_FIXED = 200.0
ACT_CYCLE = 1 / 1.2
DVE_FIXED = 270.0
DVE_CYCLE = 1 / 0.96
TT_COST = 400.0     # bf16 mask multiply (2x DVE mode)
COPY_COST = 780.0   # [65, 512] PSUM->SBUF tensor_copy on DVE
ACT_TABLE = 1300.0

_CACHE = {}


def _qtile_order():
    """(pair, qtile) processing order: starts with (0,0) (only chunk 0
    needed -> fast start) and ends with (1,0) (smallest qtile -> short
    pipeline drain). Pairs alternate so the exp engines stay fed."""
    npair = HPC // 2
    if npair != 2:
        return [(j, p) for j in range(NQT) for p in range(npair)]
    seq = [(0, 0)]
    for j in range(1, NQT):
        seq += [(j, 1), (j, 0)]
    seq += [(0, 1)]
    return seq


def _plan_from_mask(mask):
    """Classify each (qtile, kblock) region of the mask.

    Returns (plan, tiles). plan[j] is a tuple of active kblocks
    (i, kind, mi, y0, y0e): kind in {'full','mixed'}, mi indexes the
    deduped mask tiles ([KB, QT], stored doubled to [KB, 2*QT] for the
    pair layout). y0 trims the QK matmul to q-columns >= y0 (multiple of
    256, keeping fp32r's N>=256 fast mode); y0e >= y0 trims exp/mask/PV
    finer (multiple of 128 — bf16 PV has no N constraint). All columns
    < y0e are fully masked. Mixed kblocks are ordered first.
    """
    plan = []
    tiles = []
    tile_idx = {}
    for j in range(NQT):
        row = []
        mq = mask[j * QT:(j + 1) * QT]  # [QT, S] (q rows, k cols)
        for i in range(NKB):
            R = mq[:, i * KB:(i + 1) * KB]  # [QT, KB]
            if not R.any():
                continue
            if R.all():
                row.append((i, 'full', 0, 0, 0))
                continue
            live = np.nonzero(R.any(axis=1))[0]
            y0 = min((int(live[0]) // 256) * 256, QT - 256)
            y0e = min((int(live[0]) // 128) * 128, QT - 128)
            Mt = np.ascontiguousarray(R.T).astype(np.float32)  # [KB, QT]
            key = Mt.tobytes()
            if key not in tile_idx:
                tile_idx[key] = len(tiles)
                qq = np.arange(j * QT, (j + 1) * QT)[None, :]
                kk = np.arange(i * KB, (i + 1) * KB)[:, None]
                if np.array_equal(R.T, qq >= kk):
                    # on-chip generatable: keep where y - x - rel >= 0
                    tiles.append(('causal', i * KB - j * QT))
                else:
                    tiles.append(('data', Mt))
            row.append((i, 'mixed', tile_idx[key], y0, y0e))
        # mixed kblocks first: their longer chains start early and full
        # kblocks fill the pipeline behind them
        row.sort(key=lambda e: e[1] != 'mixed')
        plan.append(tuple(row))
    return tuple(plan), tiles


def _assign_engines(plan, tiles):
    """Exp-engine assignment: earliest-finish-time vs a moving PE clock.

    Each kblock's exp goes to the engine (ACT or DVE) that would finish
    it first, modeling the kblock as ready at the PE issue clock — this
    interleaves the engines (cumulative-busy-only balancing creates long
    single-engine runs that serialize against the 3-tile st lookahead).
    Mask tensor_tensors and the out copies stay on DVE.
    Returns (assign {(pair, j, idx)}, copy_assign {(pair, j, sub)}),
    values 'act' | 'dve'.
    """
    assign = {}
    copy_assign = {}
    act_free, dve_free = ACT_TABLE, 0.0
    clock = 0.0
    for j, pair in _qtile_order():
            for idx, (i, kind, mi, y0, y0e) in enumerate(plan[j]):
                cols = 2 * (QT - y0e)
                clock += cols * 0.417 + 2 * (QT - y0) * 0.417  # QK + PV
                a_fin = max(act_free, clock) + cols * ACT_CYCLE + ACT_FIXED
                d_fin = max(dve_free, clock) + cols * DVE_CYCLE + DVE_FIXED
                if d_fin < a_fin:
                    assign[(pair, j, idx)] = 'dve'
                    dve_free = d_fin
                else:
                    assign[(pair, j, idx)] = 'act'
                    act_free = a_fin
                if kind == 'mixed':
                    fin = a_fin if assign[(pair, j, idx)] == 'act' else d_fin
                    dve_free = max(dve_free, fin) + TT_COST
            for sub in range(2):
                copy_assign[(pair, j, sub)] = 'dve'
                dve_free = max(dve_free, clock) + COPY_COST
    return assign, copy_assign


def _stack_mask_tiles(tiles):
    """Stack data mask tiles doubled into bf16 [n, KB, 2*QT]."""
    import ml_dtypes
    data = [t[1] for t in tiles if t[0] == 'data']
    if not data:
        return None
    out = np.empty((len(data), KB, 2 * QT), dtype=ml_dtypes.bfloat16)
    for i, t in enumerate(data):
        tb = t.astype(ml_dtypes.bfloat16)
        out[i, :, :QT] = tb
        out[i, :, QT:] = tb
    return out


def _build(plan, tiles, repeats=1):
    from contextlib import ExitStack

    import concourse.tile as tile
    from concourse import bacc, mybir

    f32 = mybir.dt.float32
    f32r = mybir.dt.float32r
    bf16 = mybir.dt.bfloat16
    i16 = mybir.dt.int16

    assign, copy_assign = _assign_engines(plan, tiles)

    nc = bacc.Bacc("TRN2", target_bir_lowering=False, debug=False,
                   num_devices=NCORES)

    qt_d = nc.dram_tensor("qt", [HPC // 2, 128, S], f32r, kind="ExternalInput").ap()
    kt_d = nc.dram_tensor("kt", [HPC // 2, 128, S], f32r, kind="ExternalInput").ap()
    v_d = nc.dram_tensor("v", [HPC, 128, NKB * (D + 1)], bf16,
                         kind="ExternalInput").ap()
    out_d = nc.dram_tensor("out", [HPC, D + 1, S], f32, kind="ExternalOutput").ap()
    data_idx = {}  # tile index -> position in the stacked "mt" input
    for ti, t in enumerate(tiles):
        if t[0] == 'data':
            data_idx[ti] = len(data_idx)
    n_mtiles = len(data_idx)
    if n_mtiles:
        mt_d = nc.dram_tensor("mt", [n_mtiles, KB, 2 * QT], bf16,
                              kind="ExternalInput").ap()
    resident = n_mtiles <= MAX_RESIDENT_MASKS

    with tile.TileContext(nc) as tc, ExitStack() as ctx:
        qk_pool = ctx.enter_context(tc.tile_pool(name="qk", bufs=2))
        v_pool = ctx.enter_context(tc.tile_pool(name="vp", bufs=4))
        st_pool = ctx.enter_context(tc.tile_pool(name="st", bufs=3, space="PSUM"))
        pt_pool = ctx.enter_context(tc.tile_pool(name="pt", bufs=8))
        acc_pool = ctx.enter_context(tc.tile_pool(name="acc", bufs=2, space="PSUM"))
        out_pool = ctx.enter_context(tc.tile_pool(name="ob", bufs=4))
        warm_pool = ctx.enter_context(tc.tile_pool(name="wm", bufs=1))

        # prewarm the exp table on ACT while the first DMAs are in flight
        warm = warm_pool.tile([128, 8], f32, name="warm")
        nc.vector.memset(warm[:], 0.0)
        warm_o = warm_pool.tile([128, 8], bf16, name="warmo")
        nc.scalar.activation(warm_o[:], warm[:],
                             mybir.ActivationFunctionType.Exp, scale=1.0)

        m_tiles = {}
        mt_pool = ctx.enter_context(
            tc.tile_pool(name="mt", bufs=1 if resident else 2))
        # causal-pattern masks: generated on-chip once, doubled for the
        # pair layout (same pattern in both QT halves)
        for ti, t in enumerate(tiles):
            if t[0] != 'causal':
                continue
            rel = t[1]
            m = mt_pool.tile([KB, 2 * QT], bf16, tag=f"m{ti}", name=f"m{ti}")
            nc.gpsimd.memset(m[:], 1.0)
            m3g = m[:].rearrange("p (s y) -> p s y", y=QT)
            nc.gpsimd.affine_select(
                out=m3g, in_=m3g,
                compare_op=mybir.AluOpType.is_ge,
                fill=0.0, base=-rel,
                pattern=[[0, 2], [1, QT]],
                channel_multiplier=-1)
            m_tiles[ti] = m

        def _preload_masks():
            if n_mtiles and resident:
                for ti, di in data_idx.items():
                    m = mt_pool.tile([KB, 2 * QT], bf16, tag=f"m{ti}",
                                     name=f"md{ti}")
                    nc.sync.dma_start(m[:], mt_d[di])
                    m_tiles[ti] = m

        NPAIR = HPC // 2
        for rep in range(repeats):
            # chunked loads, both pairs interleaved: compute on qtile j
            # needs only k/q chunks <= j, so the first matmuls start after
            # a few chunk DMAs
            kt_c = {}
            qt_c = {}
            v_ts = {}
            out_sbs = {}
            for c in range(NQT):
                for pair in range(NPAIR):
                    kt1 = qk_pool.tile([128, QT], f32r, tag=f"kt{pair}_{c}",
                                       name=f"kt{pair}_{c}")
                    nc.sync.dma_start(kt1[:], kt_d[pair, :, c * QT:(c + 1) * QT])
                    kt_c[pair, c] = kt1
                    qt1 = qk_pool.tile([128, QT], f32r, tag=f"qt{pair}_{c}",
                                       name=f"qt{pair}_{c}")
                    nc.sync.dma_start(qt1[:], qt_d[pair, :, c * QT:(c + 1) * QT])
                    qt_c[pair, c] = qt1
                if c == 0:
                    for h in range(HPC):
                        v_t = v_pool.tile([128, NKB * (D + 1)], bf16, tag="v",
                                          name=f"v{h}")
                        nc.sync.dma_start(v_t[:], v_d[h])
                        v_ts[h] = v_t
                        out_sbs[h] = out_pool.tile([D + 1, S], f32, tag="o",
                                                   name=f"ob{h}")
                    _preload_masks()

            # pairs interleaved at qtile granularity: independent work keeps
            # the exp engines fed through each qtile's ramp-down
            for j, pair in _qtile_order():
                active = plan[j]
                accs = [acc_pool.tile([D + 1, QT], f32, tag="a",
                                      name=f"acc{pair}_{j}_{sub}")
                        for sub in range(2)]
                for idx, (i, kind, mi, y0, y0e) in enumerate(active):
                    st = st_pool.tile([128, 2 * QT], f32, tag="s",
                                      name=f"st{pair}_{j}_{idx}")
                    # both heads' QK^T back-to-back: disjoint PE row-tiles
                    # (0,0)/(64,0) -> concurrent execution
                    for sub in range(2):
                        po = 64 * sub
                        nc.tensor.matmul(
                            st[:, sub * QT + y0:(sub + 1) * QT],
                            lhsT=kt_c[pair, i // 4][po:po + 64,
                                                    (i % 4) * KB:(i % 4 + 1) * KB],
                            rhs=qt_c[pair, j][po:po + 64, y0:],
                            start=True, stop=True)
                    pt = pt_pool.tile([128, 2 * QT], bf16, tag="p",
                                      name=f"pt{pair}_{j}_{idx}")
                    if y0e == 0:
                        st_ap = st[:, :]
                        pt_ap = pt[:, :]
                    else:
                        st_ap = st[:].rearrange(
                            "p (s y) -> p s y", y=QT)[:, :, y0e:]
                        pt_ap = pt[:].rearrange(
                            "p (s y) -> p s y", y=QT)[:, :, y0e:]
                    if assign[(pair, j, idx)] == 'act':
                        nc.scalar.activation(
                            pt_ap, st_ap,
                            mybir.ActivationFunctionType.Exp,
                            scale=float(1.0 / np.sqrt(D)))
                    else:
                        nc.vector.tensor_scalar(
                            pt_ap.bitcast(i16), st_ap, A16, B16,
                            mybir.AluOpType.mult, mybir.AluOpType.add)
                    if kind == 'mixed':
                        if mi in m_tiles:
                            m_t = m_tiles[mi]
                        else:
                            m_t = mt_pool.tile([KB, 2 * QT], bf16, tag="ms")
                            nc.sync.dma_start(m_t[:], mt_d[data_idx[mi]])
                        if y0e == 0:
                            m_ap = m_t[:, :]
                        else:
                            m_ap = m_t[:].rearrange(
                                "p (s y) -> p s y", y=QT)[:, :, y0e:]
                        nc.vector.tensor_tensor(
                            pt_ap, pt_ap, m_ap, mybir.AluOpType.mult)
                    for sub in range(2):
                        nc.tensor.matmul(
                            accs[sub][:, y0e:],
                            lhsT=v_ts[2 * pair + sub][:,
                                                      i * (D + 1):(i + 1) * (D + 1)],
                            rhs=pt[:, sub * QT + y0e:(sub + 1) * QT],
                            start=(idx == 0),
                            stop=(idx == len(active) - 1))
                for sub in range(2):
                    osl = out_sbs[2 * pair + sub][:, j * QT:(j + 1) * QT]
                    if not active:
                        nc.vector.memset(osl, 0.0)
                    elif copy_assign[(pair, j, sub)] == 'act':
                        nc.scalar.activation(
                            osl, accs[sub][:],
                            mybir.ActivationFunctionType.Copy, scale=1.0)
                    else:
                        nc.vector.tensor_copy(osl, accs[sub][:])
                    nc.sync.dma_start(
                        out_d[2 * pair + sub, :, j * QT:(j + 1) * QT], osl)

    nc.compile()
    return nc


def _get_nc(mask):
    key = mask.tobytes()
    if key not in _CACHE:
        plan, mtiles = _plan_from_mask(mask)
        nc = _build(plan, mtiles)
        _CACHE[key] = (nc, mtiles)
    return _CACHE[key]


def kernel(q, k, v, mask, _trace=False):
    import ml_dtypes
    from concourse.bass_utils import run_bass_kernel_spmd

    mask = np.asarray(mask).astype(bool)
    q = np.asarray(q, dtype=np.float32).reshape(B * H, S, D)
    k = np.asarray(k, dtype=np.float32).reshape(B * H, S, D)
    v = np.asarray(v, dtype=np.float32).reshape(B * H, S, D)

    nc, mtiles = _get_nc(mask)
    mt = _stack_mask_tiles(mtiles)

    in_maps = []
    for c in range(NCORES):
        sl = slice(HPC * c, HPC * (c + 1))
        qc = np.ascontiguousarray(q[sl].transpose(0, 2, 1)).reshape(HPC // 2, 128, S)
        kc = np.ascontiguousarray(k[sl].transpose(0, 2, 1)).reshape(HPC // 2, 128, S)
        vc = np.concatenate(
            [v[sl], np.ones((HPC, S, 1), dtype=np.float32)], axis=2)
        # swizzle to the SBUF layout: [HPC, partition, kblock*(D+1)]
        vc = vc.reshape(HPC, NKB, KB, D + 1).transpose(0, 2, 1, 3)
        vc = np.ascontiguousarray(vc).reshape(HPC, KB, NKB * (D + 1))
        m = {"qt": qc, "kt": kc, "v": vc.astype(ml_dtypes.bfloat16)}
        if mt is not None:
            m["mt"] = mt
        in_maps.append(m)

    res = run_bass_kernel_spmd(nc, in_maps, core_ids=list(range(NCORES)),
                               trace=_trace)

    outs = []
    for c in range(NCORES):
        o = res.results[c]["out"]  # [HPC, D+1, S]
        num = o[:, :D, :]
        den = o[:, D:D + 1, :]
        with np.errstate(invalid='ignore', divide='ignore'):
            outs.append((num / den).transpose(0, 2, 1))  # [HPC, S, D]
    full = np.concatenate(outs, axis=0).reshape(B, H, S, D).astype(np.float32)
    if _trace:
        return full, res
    return full


# revision 35
# speedup vs baseline: 1.1840x; 1.0004x over previous
"""Causal attention kernel for Trainium2 (Bass/Tile), 8-core SPMD.

Problem: B=2, H=16, S=2048, D=64 fp32 attention with a causal mask.
Sharding: batch*heads = 32 slices -> 4 heads per core across 8 cores.

Per-core algorithm (heads processed in pairs, stacked in partitions):
  S^T = K @ Q^T computed per kblock: a [128, 2*QT] PSUM "pair tile" holds
  both heads' scores for one kblock (head0 in cols 0..QT, head1 in
  QT..2*QT). The two QK^T matmuls are issued back-to-back with lhsT at
  partition offsets 0/64, so they land on disjoint PE row-tiles
  ((0,0)/(64,0)) and execute concurrently (2x QK throughput).

  P^T = exp(S^T / 8) is split across TWO engines to break the ScalarE
  wall (exp elems/core = 8.4M at 1 col/cycle = 60us+ on ACT alone):
   - ACT kblocks: ScalarE activation, bf16 output (exact exp).
   - DVE kblocks: ONE tensor_scalar computing the Schraudolph bit-trick
     in the bf16 bit domain: int16(round(s*2^7*log2e/8 + (127-sig)*2^7))
     bitcast as bf16 == exp with ~3% ripple (well under the 2e-2 rel_err
     budget; measured end-to-end error ~1.2e-2).
  kblocks are assigned by earliest-finish-time against a moving PE
  clock (interleaves the engines AND balances them). The mask multiply
  of mixed kblocks is a bf16 tensor_tensor (2x DVE mode) against a
  resident doubled mask tile [KB, 2*QT] (causal patterns generated
  on-chip by GpSimd once at startup).

  out^T = V_aug^T @ P^T accumulated over kblocks in PSUM (V_aug bf16
  with a ones column -> row 64 of out^T is the softmax denominator).
  Host divides by the denominator and transposes back.

  QK matmuls are float32r (1 cycle/row at N>=256, so their causal trim
  y0 is 256-granular); exp/mask/PV trim finer at y0e (128-granular,
  bf16 PV has no N constraint). PSUM: 3 st pair-tiles (2 banks each) +
  2 accs = 8 banks = 3 kblocks of scheduler lookahead. Qtiles run in an
  order that starts with the smallest (fast start: one DMA chunk) and
  ends with the other pair's smallest (short pipeline drain).
"""

import sys

import numpy as np

for _p in ('/opt/trn_rl_repo', '/root/.axon_site/_ro/trn_rl_repo'):
    if _p not in sys.path:
        sys.path.append(_p)

B, H, S, D = 2, 16, 2048, 64
NCORES = 8
HPC = (B * H) // NCORES  # heads per core = 4
QT = 512                 # q tile (PSUM bank free dim)
KB = 128                 # k block (partition dim)
NQT = S // QT            # 4
NKB = S // KB            # 16
MAX_RESIDENT_MASKS = 8   # unique mask tiles kept SBUF-resident

SIGMA = 0.045
A16 = float(2**7 * np.log2(np.e) / np.sqrt(D))
B16 = float((127.0 - SIGMA) * 2**7)

# per-kblock engine cost estimates (ns) for the greedy assignment
ACT_FIXED = 200.0
ACT_CYCLE = 1 / 1.2
DVE_FIXED = 270.0
DVE_CYCLE = 1 / 1.04  # mild DVE-ward bias (HW: paired QKs leave ACT as pacer)
TT_COST = 400.0     # bf16 mask multiply (2x DVE mode)
COPY_COST = 780.0   # [65, 512] PSUM->SBUF tensor_copy on DVE
ACT_TABLE = 1300.0

_CACHE = {}


def _qtile_order():
    """(pair, qtile) processing order: starts with (0,0) (only chunk 0
    needed -> fast start) and ends with (1,0) (smallest qtile -> short
    pipeline drain). Pairs alternate so the exp engines stay fed."""
    npair = HPC // 2
    if npair != 2:
        return [(j, p) for j in range(NQT) for p in range(npair)]
    seq = [(0, 0)]
    for j in range(1, NQT):
        seq += [(j, 1), (j, 0)]
    seq += [(0, 1)]
    return seq


def _plan_from_mask(mask):
    """Classify each (qtile, kblock) region of the mask.

    Returns (plan, tiles). plan[j] is a tuple of active kblocks
    (i, kind, mi, y0, y0e): kind in {'full','mixed'}, mi indexes the
    deduped mask tiles ([KB, QT], stored doubled to [KB, 2*QT] for the
    pair layout). y0 trims the QK matmul to q-columns >= y0 (multiple of
    256, keeping fp32r's N>=256 fast mode); y0e >= y0 trims exp/mask/PV
    finer (multiple of 128 — bf16 PV has no N constraint). All columns
    < y0e are fully masked. Mixed kblocks are ordered first.
    """
    plan = []
    tiles = []
    tile_idx = {}
    for j in range(NQT):
        row = []
        mq = mask[j * QT:(j + 1) * QT]  # [QT, S] (q rows, k cols)
        for i in range(NKB):
            R = mq[:, i * KB:(i + 1) * KB]  # [QT, KB]
            if not R.any():
                continue
            if R.all():
                row.append((i, 'full', 0, 0, 0))
                continue
            live = np.nonzero(R.any(axis=1))[0]
            y0 = min((int(live[0]) // 256) * 256, QT - 256)
            y0e = min((int(live[0]) // 128) * 128, QT - 128)
            Mt = np.ascontiguousarray(R.T).astype(np.float32)  # [KB, QT]
            key = Mt.tobytes()
            if key not in tile_idx:
                tile_idx[key] = len(tiles)
                qq = np.arange(j * QT, (j + 1) * QT)[None, :]
                kk = np.arange(i * KB, (i + 1) * KB)[:, None]
                if np.array_equal(R.T, qq >= kk):
                    # on-chip generatable: keep where y - x - rel >= 0
                    tiles.append(('causal', i * KB - j * QT))
                else:
                    tiles.append(('data', Mt))
            row.append((i, 'mixed', tile_idx[key], y0, y0e))
        # mixed kblocks first: their longer chains start early and full
        # kblocks fill the pipeline behind them
        row.sort(key=lambda e: e[1] != 'mixed')
        plan.append(tuple(row))
    return tuple(plan), tiles


def _assign_engines(plan, tiles):
    """Exp-engine assignment: earliest-finish-time vs a moving PE clock.

    Each kblock's exp goes to the engine (ACT or DVE) that would finish
    it first, modeling the kblock as ready at the PE issue clock — this
    interleaves the engines (cumulative-busy-only balancing creates long
    single-engine runs that serialize against the 3-tile st lookahead).
    Mask tensor_tensors and the out copies stay on DVE.
    Returns (assign {(pair, j, idx)}, copy_assign {(pair, j, sub)}),
    values 'act' | 'dve'.
    """
    assign = {}
    copy_assign = {}
    act_free, dve_free = ACT_TABLE, 0.0
    clock = 0.0
    for j, pair in _qtile_order():
            for idx, (i, kind, mi, y0, y0e) in enumerate(plan[j]):
                cols = 2 * (QT - y0e)
                clock += cols * 0.417 + 2 * (QT - y0) * 0.417  # QK + PV
                a_fin = max(act_free, clock) + cols * ACT_CYCLE + ACT_FIXED
                d_fin = max(dve_free, clock) + cols * DVE_CYCLE + DVE_FIXED
                if d_fin < a_fin:
                    assign[(pair, j, idx)] = 'dve'
                    dve_free = d_fin
                else:
                    assign[(pair, j, idx)] = 'act'
                    act_free = a_fin
                if kind == 'mixed':
                    fin = a_fin if assign[(pair, j, idx)] == 'act' else d_fin
                    dve_free = max(dve_free, fin) + TT_COST
            for sub in range(2):
                copy_assign[(pair, j, sub)] = 'dve'
                dve_free = max(dve_free, clock) + COPY_COST
    return assign, copy_assign


def _stack_mask_tiles(tiles):
    """Stack data mask tiles doubled into bf16 [n, KB, 2*QT]."""
    import ml_dtypes
    data = [t[1] for t in tiles if t[0] == 'data']
    if not data:
        return None
    out = np.empty((len(data), KB, 2 * QT), dtype=ml_dtypes.bfloat16)
    for i, t in enumerate(data):
        tb = t.astype(ml_dtypes.bfloat16)
        out[i, :, :QT] = tb
        out[i, :, QT:] = tb
    return out


def _build(plan, tiles, repeats=1):
    from contextlib import ExitStack

    import concourse.tile as tile
    from concourse import bacc, mybir

    f32 = mybir.dt.float32
    f32r = mybir.dt.float32r
    bf16 = mybir.dt.bfloat16
    i16 = mybir.dt.int16

    assign, copy_assign = _assign_engines(plan, tiles)

    nc = bacc.Bacc("TRN2", target_bir_lowering=False, debug=False,
                   num_devices=NCORES)

    qt_d = nc.dram_tensor("qt", [HPC // 2, 128, S], f32r, kind="ExternalInput").ap()
    kt_d = nc.dram_tensor("kt", [HPC // 2, 128, S], f32r, kind="ExternalInput").ap()
    v_d = nc.dram_tensor("v", [HPC, 128, NKB * (D + 1)], bf16,
                         kind="ExternalInput").ap()
    out_d = nc.dram_tensor("out", [HPC, D + 1, S], f32, kind="ExternalOutput").ap()
    data_idx = {}  # tile index -> position in the stacked "mt" input
    for ti, t in enumerate(tiles):
        if t[0] == 'data':
            data_idx[ti] = len(data_idx)
    n_mtiles = len(data_idx)
    if n_mtiles:
        mt_d = nc.dram_tensor("mt", [n_mtiles, KB, 2 * QT], bf16,
                              kind="ExternalInput").ap()
    resident = n_mtiles <= MAX_RESIDENT_MASKS

    with tile.TileContext(nc) as tc, ExitStack() as ctx:
        qk_pool = ctx.enter_context(tc.tile_pool(name="qk", bufs=2))
        v_pool = ctx.enter_context(tc.tile_pool(name="vp", bufs=4))
        st_pool = ctx.enter_context(tc.tile_pool(name="st", bufs=3, space="PSUM"))
        pt_pool = ctx.enter_context(tc.tile_pool(name="pt", bufs=8))
        acc_pool = ctx.enter_context(tc.tile_pool(name="acc", bufs=2, space="PSUM"))
        out_pool = ctx.enter_context(tc.tile_pool(name="ob", bufs=4))
        warm_pool = ctx.enter_context(tc.tile_pool(name="wm", bufs=1))

        # prewarm the exp table on ACT while the first DMAs are in flight
        warm = warm_pool.tile([128, 8], f32, name="warm")
        nc.vector.memset(warm[:], 0.0)
        warm_o = warm_pool.tile([128, 8], bf16, name="warmo")
        nc.scalar.activation(warm_o[:], warm[:],
                             mybir.ActivationFunctionType.Exp, scale=1.0)

        m_tiles = {}
        mt_pool = ctx.enter_context(
            tc.tile_pool(name="mt", bufs=1 if resident else 2))
        # causal-pattern masks: generated on-chip once, doubled for the
        # pair layout (same pattern in both QT halves)
        for ti, t in enumerate(tiles):
            if t[0] != 'causal':
                continue
            rel = t[1]
            m = mt_pool.tile([KB, 2 * QT], bf16, tag=f"m{ti}", name=f"m{ti}")
            nc.gpsimd.memset(m[:], 1.0)
            m3g = m[:].rearrange("p (s y) -> p s y", y=QT)
            nc.gpsimd.affine_select(
                out=m3g, in_=m3g,
                compare_op=mybir.AluOpType.is_ge,
                fill=0.0, base=-rel,
                pattern=[[0, 2], [1, QT]],
                channel_multiplier=-1)
            m_tiles[ti] = m

        def _preload_masks():
            if n_mtiles and resident:
                for ti, di in data_idx.items():
                    m = mt_pool.tile([KB, 2 * QT], bf16, tag=f"m{ti}",
                                     name=f"md{ti}")
                    nc.sync.dma_start(m[:], mt_d[di])
                    m_tiles[ti] = m

        NPAIR = HPC // 2
        for rep in range(repeats):
            # chunked loads, both pairs interleaved: compute on qtile j
            # needs only k/q chunks <= j, so the first matmuls start after
            # a few chunk DMAs
            kt_c = {}
            qt_c = {}
            v_ts = {}
            out_sbs = {}
            for c in range(NQT):
                for pair in range(NPAIR):
                    kt1 = qk_pool.tile([128, QT], f32r, tag=f"kt{pair}_{c}",
                                       name=f"kt{pair}_{c}")
                    nc.sync.dma_start(kt1[:], kt_d[pair, :, c * QT:(c + 1) * QT])
                    kt_c[pair, c] = kt1
                    qt1 = qk_pool.tile([128, QT], f32r, tag=f"qt{pair}_{c}",
                                       name=f"qt{pair}_{c}")
                    nc.sync.dma_start(qt1[:], qt_d[pair, :, c * QT:(c + 1) * QT])
                    qt_c[pair, c] = qt1
                if c == 0:
                    for h in range(HPC):
                        v_t = v_pool.tile([128, NKB * (D + 1)], bf16, tag="v",
                                          name=f"v{h}")
                        nc.sync.dma_start(v_t[:], v_d[h])
                        v_ts[h] = v_t
                        out_sbs[h] = out_pool.tile([D + 1, S], f32, tag="o",
                                                   name=f"ob{h}")
                    _preload_masks()

            # pairs interleaved at qtile granularity: independent work keeps
            # the exp engines fed through each qtile's ramp-down
            for j, pair in _qtile_order():
                active = plan[j]
                accs = [acc_pool.tile([D + 1, QT], f32, tag="a",
                                      name=f"acc{pair}_{j}_{sub}")
                        for sub in range(2)]
                for idx, (i, kind, mi, y0, y0e) in enumerate(active):
                    st = st_pool.tile([128, 2 * QT], f32, tag="s",
                                      name=f"st{pair}_{j}_{idx}")
                    # both heads' QK^T back-to-back: disjoint PE row-tiles
                    # (0,0)/(64,0) -> concurrent execution
                    for sub in range(2):
                        po = 64 * sub
                        nc.tensor.matmul(
                            st[:, sub * QT + y0:(sub + 1) * QT],
                            lhsT=kt_c[pair, i // 4][po:po + 64,
                                                    (i % 4) * KB:(i % 4 + 1) * KB],
                            rhs=qt_c[pair, j][po:po + 64, y0:],
                            start=True, stop=True)
                    pt = pt_pool.tile([128, 2 * QT], bf16, tag="p",
                                      name=f"pt{pair}_{j}_{idx}")
                    if y0e == 0:
                        st_ap = st[:, :]
                        pt_ap = pt[:, :]
                    else:
                        st_ap = st[:].rearrange(
                            "p (s y) -> p s y", y=QT)[:, :, y0e:]
                        pt_ap = pt[:].rearrange(
                            "p (s y) -> p s y", y=QT)[:, :, y0e:]
                    if assign[(pair, j, idx)] == 'act':
                        nc.scalar.activation(
                            pt_ap, st_ap,
                            mybir.ActivationFunctionType.Exp,
                            scale=float(1.0 / np.sqrt(D)))
                    else:
                        nc.vector.tensor_scalar(
                            pt_ap.bitcast(i16), st_ap, A16, B16,
                            mybir.AluOpType.mult, mybir.AluOpType.add)
                    if kind == 'mixed':
                        if mi in m_tiles:
                            m_t = m_tiles[mi]
                        else:
                            m_t = mt_pool.tile([KB, 2 * QT], bf16, tag="ms")
                            nc.sync.dma_start(m_t[:], mt_d[data_idx[mi]])
                        if y0e == 0:
                            m_ap = m_t[:, :]
                        else:
                            m_ap = m_t[:].rearrange(
                                "p (s y) -> p s y", y=QT)[:, :, y0e:]
                        nc.vector.tensor_tensor(
                            pt_ap, pt_ap, m_ap, mybir.AluOpType.mult)
                    for sub in range(2):
                        nc.tensor.matmul(
                            accs[sub][:, y0e:],
                            lhsT=v_ts[2 * pair + sub][:,
                                                      i * (D + 1):(i + 1) * (D + 1)],
                            rhs=pt[:, sub * QT + y0e:(sub + 1) * QT],
                            start=(idx == 0),
                            stop=(idx == len(active) - 1))
                for sub in range(2):
                    osl = out_sbs[2 * pair + sub][:, j * QT:(j + 1) * QT]
                    if not active:
                        nc.vector.memset(osl, 0.0)
                    elif copy_assign[(pair, j, sub)] == 'act':
                        nc.scalar.activation(
                            osl, accs[sub][:],
                            mybir.ActivationFunctionType.Copy, scale=1.0)
                    else:
                        nc.vector.tensor_copy(osl, accs[sub][:])
                    nc.sync.dma_start(
                        out_d[2 * pair + sub, :, j * QT:(j + 1) * QT], osl)

    nc.compile()
    return nc


def _get_nc(mask):
    key = mask.tobytes()
    if key not in _CACHE:
        plan, mtiles = _plan_from_mask(mask)
        nc = _build(plan, mtiles)
        _CACHE[key] = (nc, mtiles)
    return _CACHE[key]


def kernel(q, k, v, mask, _trace=False):
    import ml_dtypes
    from concourse.bass_utils import run_bass_kernel_spmd

    mask = np.asarray(mask).astype(bool)
    q = np.asarray(q, dtype=np.float32).reshape(B * H, S, D)
    k = np.asarray(k, dtype=np.float32).reshape(B * H, S, D)
    v = np.asarray(v, dtype=np.float32).reshape(B * H, S, D)

    nc, mtiles = _get_nc(mask)
    mt = _stack_mask_tiles(mtiles)

    in_maps = []
    for c in range(NCORES):
        sl = slice(HPC * c, HPC * (c + 1))
        qc = np.ascontiguousarray(q[sl].transpose(0, 2, 1)).reshape(HPC // 2, 128, S)
        kc = np.ascontiguousarray(k[sl].transpose(0, 2, 1)).reshape(HPC // 2, 128, S)
        vc = np.concatenate(
            [v[sl], np.ones((HPC, S, 1), dtype=np.float32)], axis=2)
        # swizzle to the SBUF layout: [HPC, partition, kblock*(D+1)]
        vc = vc.reshape(HPC, NKB, KB, D + 1).transpose(0, 2, 1, 3)
        vc = np.ascontiguousarray(vc).reshape(HPC, KB, NKB * (D + 1))
        m = {"qt": qc, "kt": kc, "v": vc.astype(ml_dtypes.bfloat16)}
        if mt is not None:
            m["mt"] = mt
        in_maps.append(m)

    res = run_bass_kernel_spmd(nc, in_maps, core_ids=list(range(NCORES)),
                               trace=_trace)

    outs = []
    for c in range(NCORES):
        o = res.results[c]["out"]  # [HPC, D+1, S]
        num = o[:, :D, :]
        den = o[:, D:D + 1, :]
        with np.errstate(invalid='ignore', divide='ignore'):
            outs.append((num / den).transpose(0, 2, 1))  # [HPC, S, D]
    full = np.concatenate(outs, axis=0).reshape(B, H, S, D).astype(np.float32)
    if _trace:
        return full, res
    return full
